# revision 2
# baseline (speedup 1.0000x reference)
"""Trainium2 Bass kernel for the stacked-Chebyshev locally-connected net.

Reference computation (B=256, k=6250, d*d=4096, O=10):
    x1 = z @ (mask*T1).T
    x2 = 2*(z @ (mask*T2).T)*x1 - T0
    x3 = 2*(z @ (mask*T3).T)*x2 - x1
    out = x3 @ C_w.T + C_b

The mask is a locally-connected conv pattern: 16x16 patch, stride 2, 25x25
positions, stacked 10x.  Rows that share the same patch-row index i have a
single contiguous, 128-aligned 1024-wide support in d — grouping by i cuts
the matmul contraction from 4096 to 1024 (4x fewer MACs than dense).

Sharding: 25 i-groups -> 8 cores get whole consecutive groups (4,3,3,...).
Each core runs 4 uniform "slots" (one group each, zero-padded slots where the
core has only 3 real groups).  A slot is 250 k-columns split into 2 k-tiles
of 128 (125 real), contracted over 8 K-chunks against a shared 11-chunk z.T
window.  The Chebyshev recurrence is elementwise in [k, B] layout (T0 is a
per-partition scalar), and the k->O projection accumulates in PSUM per core;
per-core partials are summed on the host (the "reduce" of the k-sharding).
"""

import numpy as np

import concourse.bass as bass
import concourse.mybir as mybir
import concourse.tile as tile
from concourse import bacc
from concourse.bass_utils import run_bass_kernel_spmd

F32 = mybir.dt.float32
F32R = mybir.dt.float32r

B = 256          # batch
O = 10           # output classes
D2 = 4096        # d*d
N_CORES = 8
SLOTS = 4        # uniform slots per core
SLOT_ROWS = 1024  # d-window rows per slot (8 K-chunks)
SLOT_COLS = 256   # 2 k-tiles of 128 (125 real cols each)
WIN_CH = 11      # z.T window chunks per core (slot s uses chunks s..s+7)
QUOTA = (4, 3, 3, 3, 3, 3, 3, 3)   # real groups per core (sum = 25)
I0 = (0, 4, 7, 10, 13, 16, 19, 22)  # first group index per core

# matmul operand dtype: float32r streams at 1 cyc/row for N>=256 (vs 4 for
# plain float32) at fp32 storage; flip to F32 if accuracy ever demands it.
MM_DT = F32R


def _group_cols(i):
    """k-column indices of patch-row-group i (order: stack-major, then j)."""
    return np.array(
        [s * 625 + i * 25 + j for s in range(10) for j in range(25)], dtype=np.int64
    )


def _build_nc():
    nc = bacc.Bacc(
        "TRN2", target_bir_lowering=False, debug=False, num_devices=N_CORES
    )
    zw = nc.dram_tensor("zw", [WIN_CH * 128, B], MM_DT, kind="ExternalInput").ap()
    w_dram = [
        nc.dram_tensor(f"w{l}", [SLOTS * SLOT_ROWS, SLOT_COLS], MM_DT,
                       kind="ExternalInput").ap()
        for l in (1, 2, 3)
    ]
    # negated T0 (used as an additive bias on the scalar engine)
    t0n = nc.dram_tensor("t0n", [SLOTS * 256, 1], F32, kind="ExternalInput").ap()
    cwt = nc.dram_tensor("cwt", [SLOTS * 256, O], MM_DT, kind="ExternalInput").ap()
    out = nc.dram_tensor("out", [O, B], F32, kind="ExternalOutput").ap()

    with tile.TileContext(nc) as tc:
        with (
            tc.tile_pool(name="zpool", bufs=1) as zpool,
            tc.tile_pool(name="cpool", bufs=1) as cpool,
            tc.tile_pool(name="wpool", bufs=12) as wpool,
            tc.tile_pool(name="xpool", bufs=3) as xpool,
            tc.tile_pool(name="ppool", bufs=6, space="PSUM") as ppool,
            tc.tile_pool(name="opool", bufs=1, space="PSUM") as opool,
        ):
            # resident z.T window: chunk c lives at free-columns [c*B, (c+1)*B)
            zt = zpool.tile([128, WIN_CH * B], MM_DT)
            for c in range(WIN_CH):
                nc.sync.dma_start(zt[:, c * B:(c + 1) * B],
                                  zw[c * 128:(c + 1) * 128, :])

            t0_sb = cpool.tile([128, 2 * SLOTS], F32, tag="t0")
            cw_sb = cpool.tile([128, 2 * SLOTS * O], MM_DT, tag="cw")
            for s in range(SLOTS):
                for kt in range(2):
                    r0 = s * 256 + kt * 128
                    col = 2 * s + kt
                    nc.sync.dma_start(t0_sb[:, col:col + 1], t0n[r0:r0 + 128, :])
                    nc.sync.dma_start(cw_sb[:, col * O:(col + 1) * O],
                                      cwt[r0:r0 + 128, :])

            psum_o = opool.tile([O, B], F32)
            n_proj = 0

            for s in range(SLOTS):
                psums = []
                for li in range(3):
                    pa = ppool.tile([128, B], F32, tag="ps")
                    pb = ppool.tile([128, B], F32, tag="ps")
                    for kc in range(8):
                        w = wpool.tile([128, SLOT_COLS], MM_DT, tag="w")
                        nc.sync.dma_start(
                            w[:],
                            w_dram[li][s * SLOT_ROWS + kc * 128:
                                       s * SLOT_ROWS + (kc + 1) * 128, :])
                        zc = zt[:, (s + kc) * B:(s + kc + 1) * B]
                        nc.tensor.matmul(pa[:], w[:, 0:128], zc,
                                         start=(kc == 0), stop=(kc == 7))
                        nc.tensor.matmul(pb[:], w[:, 128:256], zc,
                                         start=(kc == 0), stop=(kc == 7))
                    psums.append((pa, pb))

                for kt in range(2):
                    p1 = psums[0][kt]
                    p2 = psums[1][kt]
                    p3 = psums[2][kt]
                    col = 2 * s + kt
                    x1 = xpool.tile([128, B], F32, tag="x1")
                    m2 = xpool.tile([128, B], F32, tag="m2")
                    x2 = xpool.tile([128, B], F32, tag="x2")
                    m3 = xpool.tile([128, B], F32, tag="m3")
                    x3 = xpool.tile([128, B], MM_DT, tag="x3")
                    # scalar engine: x1 evac + bias-add; vector engine: muls/sub
                    nc.scalar.copy(x1[:], p1[:])
                    nc.vector.tensor_mul(m2[:], p2[:], x1[:])
                    nc.scalar.activation(x2[:], m2[:],
                                         mybir.ActivationFunctionType.Identity,
                                         bias=t0_sb[:, col:col + 1])
                    nc.vector.tensor_mul(m3[:], p3[:], x2[:])
                    nc.vector.tensor_sub(x3[:], m3[:], x1[:])
                    n_proj += 1
                    nc.tensor.matmul(psum_o[:],
                                     cw_sb[:, col * O:(col + 1) * O],
                                     x3[:],
                                     start=(n_proj == 1),
                                     stop=(n_proj == 2 * SLOTS))

            out_sb = cpool.tile([O, B], F32, tag="out")
            nc.vector.tensor_copy(out_sb[:], psum_o[:])
            nc.sync.dma_start(out[:], out_sb[:])

    nc.compile()
    return nc


_NC = None


def _get_nc():
    global _NC
    if _NC is None:
        _NC = _build_nc()
    return _NC


def _prepare_in_maps(z, T1, T2, T3, T0, C_w, mask):
    z = np.ascontiguousarray(np.asarray(z, dtype=np.float32).reshape(B, D2))
    T1 = np.asarray(T1, dtype=np.float32)
    T2 = np.asarray(T2, dtype=np.float32)
    T3 = np.asarray(T3, dtype=np.float32)
    T0 = np.asarray(T0, dtype=np.float32)
    C_w = np.asarray(C_w, dtype=np.float32)
    mask = np.asarray(mask, dtype=np.float32)

    zT_pad = np.zeros((128 * I0[-1] + WIN_CH * 128, B), np.float32)
    zT_pad[:D2] = z.T
    Ts = (T1, T2, T3)
    scales = (1.0, 2.0, 2.0)

    in_maps = []
    for c in range(N_CORES):
        i0 = I0[c]
        m = {"zw": np.ascontiguousarray(zT_pad[128 * i0: 128 * i0 + WIN_CH * 128])}
        wts = {l: np.zeros((SLOTS * SLOT_ROWS, SLOT_COLS), np.float32)
               for l in (1, 2, 3)}
        t0n = np.zeros((SLOTS * 256, 1), np.float32)
        cwt = np.zeros((SLOTS * 256, O), np.float32)
        for s in range(QUOTA[c]):
            g = i0 + s
            cols = _group_cols(g)
            dwin = np.arange(128 * g, 128 * g + 1024)
            ix = np.ix_(cols, dwin)
            mwin = mask[ix]
            for li, (T, sc) in enumerate(zip(Ts, scales)):
                AT = (sc * T[ix] * mwin).T          # [1024, 250]
                blk = wts[li + 1][s * SLOT_ROWS:(s + 1) * SLOT_ROWS]
                blk[:, 0:125] = AT[:, 0:125]
                blk[:, 128:253] = AT[:, 125:250]
            t0n[s * 256 + 0: s * 256 + 125, 0] = -T0[cols[0:125]]
            t0n[s * 256 + 128: s * 256 + 253, 0] = -T0[cols[125:250]]
            cwt[s * 256 + 0: s * 256 + 125] = C_w[:, cols[0:125]].T
            cwt[s * 256 + 128: s * 256 + 253] = C_w[:, cols[125:250]].T
        m["w1"], m["w2"], m["w3"] = wts[1], wts[2], wts[3]
        m["t0n"] = t0n
        m["cwt"] = cwt
        in_maps.append(m)
    return in_maps


def kernel(z, T1, T2, T3, T0, C_w, C_b, mask):
    nc = _get_nc()
    in_maps = _prepare_in_maps(z, T1, T2, T3, T0, C_w, mask)
    res = run_bass_kernel_spmd(nc, in_maps, core_ids=list(range(N_CORES)))
    total = np.zeros((O, B), np.float32)
    for c in range(N_CORES):
        total += res.results[c]["out"]
    C_b = np.asarray(C_b, dtype=np.float32)
    return (total.T + C_b).astype(np.float32)


# revision 4
# speedup vs baseline: 1.3433x; 1.3433x over previous
"""Trainium2 Bass kernel for the stacked-Chebyshev locally-connected net.

Reference computation (B=256, k=6250, d*d=4096, O=10):
    x1 = z @ (mask*T1).T
    x2 = 2*(z @ (mask*T2).T)*x1 - T0
    x3 = 2*(z @ (mask*T3).T)*x2 - x1
    out = x3 @ C_w.T + C_b

The mask is a locally-connected conv pattern: 16x16 patch, stride 2, 25x25
positions, stacked 10x.  Rows that share the same patch-row index i have a
single contiguous, 128-aligned 1024-wide support in d — grouping by i cuts
the matmul contraction from 4096 to 1024 (4x fewer MACs than dense).

Sharding: 25 i-groups -> 8 cores get whole consecutive groups (4,3,3,...).
Each core runs 4 uniform "slots" (one group each, zero-padded slots where the
core has only 3 real groups).  A slot is 250 k-columns split into 2 k-tiles
of 128 (125 real), contracted over 8 K-chunks against a shared 11-chunk z.T
window.  The Chebyshev recurrence is elementwise in [k, B] layout (T0 is a
per-partition scalar), and the k->O projection accumulates in PSUM per core;
per-core partials are summed on the host (the "reduce" of the k-sharding).
"""

import numpy as np

import concourse.bass as bass
import concourse.mybir as mybir
import concourse.tile as tile
from concourse import bacc
from concourse.bass_utils import run_bass_kernel_spmd

F32 = mybir.dt.float32
F32R = mybir.dt.float32r

B = 256          # batch
O = 10           # output classes
D2 = 4096        # d*d
N_CORES = 8
SLOTS = 4        # uniform slots per core
SLOT_ROWS = 1024  # d-window rows per slot (8 K-chunks)
SLOT_COLS = 256   # 2 k-tiles of 128 (125 real cols each)
WIN_CH = 11      # z.T window chunks per core (slot s uses chunks s..s+7)
QUOTA = (4, 3, 3, 3, 3, 3, 3, 3)   # real groups per core (sum = 25)
I0 = (0, 4, 7, 10, 13, 16, 19, 22)  # first group index per core

# matmul operand dtype: float32r streams at 1 cyc/row for N>=256 (vs 4 for
# plain float32) at fp32 storage; flip to F32 if accuracy ever demands it.
MM_DT = F32R


def _group_cols(i):
    """k-column indices of patch-row-group i (order: stack-major, then j)."""
    return np.array(
        [s * 625 + i * 25 + j for s in range(10) for j in range(25)], dtype=np.int64
    )


def _build_nc():
    nc = bacc.Bacc(
        "TRN2", target_bir_lowering=False, debug=False, num_devices=N_CORES
    )
    zw = nc.dram_tensor("zw", [WIN_CH * 128, B], MM_DT, kind="ExternalInput").ap()
    w_dram = [
        nc.dram_tensor(f"w{l}", [SLOTS * SLOT_ROWS, SLOT_COLS], MM_DT,
                       kind="ExternalInput").ap()
        for l in (1, 2, 3)
    ]
    # negated T0 (used as an additive bias on the scalar engine)
    t0n = nc.dram_tensor("t0n", [SLOTS * 256, 1], F32, kind="ExternalInput").ap()
    cwt = nc.dram_tensor("cwt", [SLOTS * 256, O], MM_DT, kind="ExternalInput").ap()
    out = nc.dram_tensor("out", [O, B], F32, kind="ExternalOutput").ap()

    with tile.TileContext(nc) as tc:
        with (
            tc.tile_pool(name="zpool", bufs=1) as zpool,
            tc.tile_pool(name="cpool", bufs=1) as cpool,
            tc.tile_pool(name="wpool", bufs=6) as wpool,
            tc.tile_pool(name="xpool", bufs=3) as xpool,
            tc.tile_pool(name="ppool", bufs=7, space="PSUM") as ppool,
            tc.tile_pool(name="opool", bufs=1, space="PSUM") as opool,
        ):
            # resident z.T window: chunk c lives at free-columns [c*B, (c+1)*B)
            zt = zpool.tile([128, WIN_CH * B], MM_DT)
            zw_r = zw.rearrange("(c p) n -> p c n", p=128)
            zt_r = zt[:].rearrange("p (c n) -> p c n", n=B)
            h = WIN_CH // 2
            nc.sync.dma_start(zt_r[:, 0:h], zw_r[:, 0:h])
            nc.scalar.dma_start(zt_r[:, h:], zw_r[:, h:])

            t0_sb = cpool.tile([128, 2 * SLOTS], F32, tag="t0")
            cw_sb = cpool.tile([128, 2 * SLOTS * O], MM_DT, tag="cw")
            nc.sync.dma_start(t0_sb[:].rearrange("p (c n) -> p c n", n=1),
                              t0n.rearrange("(c p) n -> p c n", p=128))
            nc.scalar.dma_start(cw_sb[:].rearrange("p (c n) -> p c n", n=O),
                                cwt.rearrange("(c p) n -> p c n", p=128))

            psum_o = opool.tile([O, B], F32)
            n_proj = 0

            for s in range(SLOTS):
                x1s = {}
                x3s = {}
                for li in range(3):
                    # one big DMA per (slot, layer): [128, 8*256] with the 8
                    # K-chunks side by side in the free dim
                    w = wpool.tile([128, 8 * SLOT_COLS], MM_DT, tag="w")
                    src = w_dram[li][s * SLOT_ROWS:(s + 1) * SLOT_ROWS, :]
                    eng = (nc.sync, nc.scalar)[(s * 3 + li) % 2]
                    eng.dma_start(w[:].rearrange("p (c n) -> p c n", n=SLOT_COLS),
                                  src.rearrange("(c p) n -> p c n", p=128))

                    pa = ppool.tile([128, B], F32, tag="ps")
                    pb = ppool.tile([128, B], F32, tag="ps")
                    for kc in range(8):
                        zc = zt[:, (s + kc) * B:(s + kc + 1) * B]
                        w0 = w[:, kc * SLOT_COLS:kc * SLOT_COLS + 128]
                        w1 = w[:, kc * SLOT_COLS + 128:(kc + 1) * SLOT_COLS]
                        nc.tensor.matmul(pa[:], w0, zc,
                                         start=(kc == 0), stop=(kc == 7))
                        nc.tensor.matmul(pb[:], w1, zc,
                                         start=(kc == 0), stop=(kc == 7))

                    # evacuate/combine right after each layer so the psum
                    # pair frees early and PE can run ahead
                    for kt, p in enumerate((pa, pb)):
                        col = 2 * s + kt
                        if li == 0:
                            x1 = xpool.tile([128, B], F32, tag="x1")
                            nc.scalar.copy(x1[:], p[:])
                            x1s[kt] = x1
                        elif li == 1:
                            m2 = xpool.tile([128, B], F32, tag="m2")
                            x2 = xpool.tile([128, B], F32, tag="x2")
                            nc.vector.tensor_mul(m2[:], p[:], x1s[kt][:])
                            nc.scalar.activation(
                                x2[:], m2[:],
                                mybir.ActivationFunctionType.Identity,
                                bias=t0_sb[:, col:col + 1])
                            x1s[kt + 2] = x2
                        else:
                            m3 = xpool.tile([128, B], F32, tag="m3")
                            x3 = xpool.tile([128, B], MM_DT, tag="x3")
                            nc.vector.tensor_mul(m3[:], p[:], x1s[kt + 2][:])
                            nc.vector.tensor_sub(x3[:], m3[:], x1s[kt][:])
                            x3s[kt] = x3

                for kt in range(2):
                    col = 2 * s + kt
                    n_proj += 1
                    nc.tensor.matmul(psum_o[:],
                                     cw_sb[:, col * O:(col + 1) * O],
                                     x3s[kt][:],
                                     start=(n_proj == 1),
                                     stop=(n_proj == 2 * SLOTS))

            out_sb = cpool.tile([O, B], F32, tag="out")
            nc.vector.tensor_copy(out_sb[:], psum_o[:])
            nc.sync.dma_start(out[:], out_sb[:])

    nc.compile()
    return nc


_NC = None


def _get_nc():
    global _NC
    if _NC is None:
        _NC = _build_nc()
    return _NC


def _prepare_in_maps(z, T1, T2, T3, T0, C_w, mask):
    z = np.ascontiguousarray(np.asarray(z, dtype=np.float32).reshape(B, D2))
    T1 = np.asarray(T1, dtype=np.float32)
    T2 = np.asarray(T2, dtype=np.float32)
    T3 = np.asarray(T3, dtype=np.float32)
    T0 = np.asarray(T0, dtype=np.float32)
    C_w = np.asarray(C_w, dtype=np.float32)
    mask = np.asarray(mask, dtype=np.float32)

    zT_pad = np.zeros((128 * I0[-1] + WIN_CH * 128, B), np.float32)
    zT_pad[:D2] = z.T
    Ts = (T1, T2, T3)
    scales = (1.0, 2.0, 2.0)

    in_maps = []
    for c in range(N_CORES):
        i0 = I0[c]
        m = {"zw": np.ascontiguousarray(zT_pad[128 * i0: 128 * i0 + WIN_CH * 128])}
        wts = {l: np.zeros((SLOTS * SLOT_ROWS, SLOT_COLS), np.float32)
               for l in (1, 2, 3)}
        t0n = np.zeros((SLOTS * 256, 1), np.float32)
        cwt = np.zeros((SLOTS * 256, O), np.float32)
        for s in range(QUOTA[c]):
            g = i0 + s
            cols = _group_cols(g)
            dwin = np.arange(128 * g, 128 * g + 1024)
            ix = np.ix_(cols, dwin)
            mwin = mask[ix]
            for li, (T, sc) in enumerate(zip(Ts, scales)):
                AT = (sc * T[ix] * mwin).T          # [1024, 250]
                blk = wts[li + 1][s * SLOT_ROWS:(s + 1) * SLOT_ROWS]
                blk[:, 0:125] = AT[:, 0:125]
                blk[:, 128:253] = AT[:, 125:250]
            t0n[s * 256 + 0: s * 256 + 125, 0] = -T0[cols[0:125]]
            t0n[s * 256 + 128: s * 256 + 253, 0] = -T0[cols[125:250]]
            cwt[s * 256 + 0: s * 256 + 125] = C_w[:, cols[0:125]].T
            cwt[s * 256 + 128: s * 256 + 253] = C_w[:, cols[125:250]].T
        m["w1"], m["w2"], m["w3"] = wts[1], wts[2], wts[3]
        m["t0n"] = t0n
        m["cwt"] = cwt
        in_maps.append(m)
    return in_maps


def kernel(z, T1, T2, T3, T0, C_w, C_b, mask):
    nc = _get_nc()
    in_maps = _prepare_in_maps(z, T1, T2, T3, T0, C_w, mask)
    res = run_bass_kernel_spmd(nc, in_maps, core_ids=list(range(N_CORES)))
    total = np.zeros((O, B), np.float32)
    for c in range(N_CORES):
        total += res.results[c]["out"]
    C_b = np.asarray(C_b, dtype=np.float32)
    return (total.T + C_b).astype(np.float32)


# revision 5
# speedup vs baseline: 1.5952x; 1.1876x over previous
"""Trainium2 Bass kernel for the stacked-Chebyshev locally-connected net.

Reference computation (B=256, k=6250, d*d=4096, O=10):
    x1 = z @ (mask*T1).T
    x2 = 2*(z @ (mask*T2).T)*x1 - T0
    x3 = 2*(z @ (mask*T3).T)*x2 - x1
    out = x3 @ C_w.T + C_b

The mask is a locally-connected conv pattern: 16x16 patch, stride 2, 25x25
positions, stacked 10x.  Rows that share the same patch-row index i have a
single contiguous, 128-aligned 1024-wide support in d — grouping by i cuts
the matmul contraction from 4096 to 1024 (4x fewer MACs than dense).

Sharding: 25 i-groups -> 8 cores get whole consecutive groups (4,3,3,...).
Each core runs 4 uniform "slots" (one group each, zero-padded slots where the
core has only 3 real groups).  A slot is 250 k-columns split into 2 k-tiles
of 128 (125 real), contracted over 8 K-chunks against a shared 11-chunk z.T
window.  The Chebyshev recurrence is elementwise in [k, B] layout (T0 is a
per-partition scalar), and the k->O projection accumulates in PSUM per core;
per-core partials are summed on the host (the "reduce" of the k-sharding).
"""

import numpy as np

import concourse.bass as bass
import concourse.mybir as mybir
import concourse.tile as tile
from concourse import bacc
from concourse.bass_utils import run_bass_kernel_spmd

F32 = mybir.dt.float32
F32R = mybir.dt.float32r

B = 256          # batch
O = 10           # output classes
D2 = 4096        # d*d
N_CORES = 8
SLOTS = 4        # uniform slots per core
SLOT_ROWS = 1024  # d-window rows per slot (8 K-chunks)
SLOT_COLS = 256   # 2 k-tiles of 128 (125 real cols each)
WIN_CH = 11      # z.T window chunks per core (slot s uses chunks s..s+7)
QUOTA = (4, 3, 3, 3, 3, 3, 3, 3)   # real groups per core (sum = 25)
I0 = (0, 4, 7, 10, 13, 16, 19, 22)  # first group index per core

# matmul operand dtype: float32r streams at 1 cyc/row for N>=256 (vs 4 for
# plain float32) at fp32 storage; flip to F32 if accuracy ever demands it.
MM_DT = F32R


def _group_cols(i):
    """k-column indices of patch-row-group i (order: stack-major, then j)."""
    return np.array(
        [s * 625 + i * 25 + j for s in range(10) for j in range(25)], dtype=np.int64
    )


def _build_nc():
    nc = bacc.Bacc(
        "TRN2", target_bir_lowering=False, debug=False, num_devices=N_CORES
    )
    # all inputs are host-pre-transposed into exact SBUF layout so every
    # DMA is a plain contiguous 2D copy (cheap descriptors, big packets)
    zw = nc.dram_tensor("zw", [128, WIN_CH * B], MM_DT, kind="ExternalInput").ap()
    w_dram = [
        nc.dram_tensor(f"w{l}", [SLOTS * 128, 8 * SLOT_COLS], MM_DT,
                       kind="ExternalInput").ap()
        for l in (1, 2, 3)
    ]
    # negated T0 (used as an additive bias on the scalar engine)
    t0n = nc.dram_tensor("t0n", [128, 2 * SLOTS], F32, kind="ExternalInput").ap()
    cwt = nc.dram_tensor("cwt", [128, 2 * SLOTS * O], MM_DT, kind="ExternalInput").ap()
    out = nc.dram_tensor("out", [O, B], F32, kind="ExternalOutput").ap()

    with tile.TileContext(nc) as tc:
        with (
            tc.tile_pool(name="zpool", bufs=1) as zpool,
            tc.tile_pool(name="cpool", bufs=1) as cpool,
            tc.tile_pool(name="wpool", bufs=6) as wpool,
            tc.tile_pool(name="xpool", bufs=3) as xpool,
            tc.tile_pool(name="ppool", bufs=7, space="PSUM") as ppool,
            tc.tile_pool(name="opool", bufs=1, space="PSUM") as opool,
        ):
            # resident z.T window: chunk c lives at free-columns [c*B, (c+1)*B)
            zt = zpool.tile([128, WIN_CH * B], MM_DT)
            h = (WIN_CH * B) // 2
            nc.sync.dma_start(zt[:, 0:h], zw[:, 0:h])
            nc.scalar.dma_start(zt[:, h:], zw[:, h:])

            t0_sb = cpool.tile([128, 2 * SLOTS], F32, tag="t0")
            cw_sb = cpool.tile([128, 2 * SLOTS * O], MM_DT, tag="cw")
            nc.sync.dma_start(t0_sb[:], t0n[:])
            nc.scalar.dma_start(cw_sb[:], cwt[:])

            psum_o = opool.tile([O, B], F32)
            n_proj = 0

            for s in range(SLOTS):
                x1s = {}
                x3s = {}
                for li in range(3):
                    # one big DMA per (slot, layer): [128, 8*256] with the 8
                    # K-chunks side by side in the free dim
                    w = wpool.tile([128, 8 * SLOT_COLS], MM_DT, tag="w")
                    eng = (nc.sync, nc.scalar)[(s * 3 + li) % 2]
                    eng.dma_start(w[:], w_dram[li][s * 128:(s + 1) * 128, :])

                    pa = ppool.tile([128, B], F32, tag="ps")
                    pb = ppool.tile([128, B], F32, tag="ps")
                    for kc in range(8):
                        zc = zt[:, (s + kc) * B:(s + kc + 1) * B]
                        w0 = w[:, kc * SLOT_COLS:kc * SLOT_COLS + 128]
                        w1 = w[:, kc * SLOT_COLS + 128:(kc + 1) * SLOT_COLS]
                        nc.tensor.matmul(pa[:], w0, zc,
                                         start=(kc == 0), stop=(kc == 7))
                        nc.tensor.matmul(pb[:], w1, zc,
                                         start=(kc == 0), stop=(kc == 7))

                    # evacuate/combine right after each layer so the psum
                    # pair frees early and PE can run ahead
                    for kt, p in enumerate((pa, pb)):
                        col = 2 * s + kt
                        if li == 0:
                            x1 = xpool.tile([128, B], F32, tag="x1")
                            nc.scalar.copy(x1[:], p[:])
                            x1s[kt] = x1
                        elif li == 1:
                            m2 = xpool.tile([128, B], F32, tag="m2")
                            x2 = xpool.tile([128, B], F32, tag="x2")
                            nc.vector.tensor_mul(m2[:], p[:], x1s[kt][:])
                            nc.scalar.activation(
                                x2[:], m2[:],
                                mybir.ActivationFunctionType.Identity,
                                bias=t0_sb[:, col:col + 1])
                            x1s[kt + 2] = x2
                        else:
                            m3 = xpool.tile([128, B], F32, tag="m3")
                            x3 = xpool.tile([128, B], MM_DT, tag="x3")
                            nc.vector.tensor_mul(m3[:], p[:], x1s[kt + 2][:])
                            nc.vector.tensor_sub(x3[:], m3[:], x1s[kt][:])
                            x3s[kt] = x3

                for kt in range(2):
                    col = 2 * s + kt
                    n_proj += 1
                    nc.tensor.matmul(psum_o[:],
                                     cw_sb[:, col * O:(col + 1) * O],
                                     x3s[kt][:],
                                     start=(n_proj == 1),
                                     stop=(n_proj == 2 * SLOTS))

            out_sb = cpool.tile([O, B], F32, tag="out")
            nc.vector.tensor_copy(out_sb[:], psum_o[:])
            nc.sync.dma_start(out[:], out_sb[:])

    nc.compile()
    return nc


_NC = None


def _get_nc():
    global _NC
    if _NC is None:
        _NC = _build_nc()
    return _NC


def _prepare_in_maps(z, T1, T2, T3, T0, C_w, mask):
    z = np.ascontiguousarray(np.asarray(z, dtype=np.float32).reshape(B, D2))
    T1 = np.asarray(T1, dtype=np.float32)
    T2 = np.asarray(T2, dtype=np.float32)
    T3 = np.asarray(T3, dtype=np.float32)
    T0 = np.asarray(T0, dtype=np.float32)
    C_w = np.asarray(C_w, dtype=np.float32)
    mask = np.asarray(mask, dtype=np.float32)

    zT_pad = np.zeros((128 * I0[-1] + WIN_CH * 128, B), np.float32)
    zT_pad[:D2] = z.T
    Ts = (T1, T2, T3)
    scales = (1.0, 2.0, 2.0)

    in_maps = []
    for c in range(N_CORES):
        i0 = I0[c]
        zwin = zT_pad[128 * i0: 128 * i0 + WIN_CH * 128]
        m = {"zw": np.ascontiguousarray(
            zwin.reshape(WIN_CH, 128, B).transpose(1, 0, 2).reshape(128, WIN_CH * B))}
        wts = {l: np.zeros((SLOTS * SLOT_ROWS, SLOT_COLS), np.float32)
               for l in (1, 2, 3)}
        t0n = np.zeros((SLOTS * 256, 1), np.float32)
        cwt = np.zeros((SLOTS * 256, O), np.float32)
        for s in range(QUOTA[c]):
            g = i0 + s
            cols = _group_cols(g)
            dwin = np.arange(128 * g, 128 * g + 1024)
            ix = np.ix_(cols, dwin)
            mwin = mask[ix]
            for li, (T, sc) in enumerate(zip(Ts, scales)):
                AT = (sc * T[ix] * mwin).T          # [1024, 250]
                blk = wts[li + 1][s * SLOT_ROWS:(s + 1) * SLOT_ROWS]
                blk[:, 0:125] = AT[:, 0:125]
                blk[:, 128:253] = AT[:, 125:250]
            t0n[s * 256 + 0: s * 256 + 125, 0] = -T0[cols[0:125]]
            t0n[s * 256 + 128: s * 256 + 253, 0] = -T0[cols[125:250]]
            cwt[s * 256 + 0: s * 256 + 125] = C_w[:, cols[0:125]].T
            cwt[s * 256 + 128: s * 256 + 253] = C_w[:, cols[125:250]].T
        for l in (1, 2, 3):
            m[f"w{l}"] = np.ascontiguousarray(
                wts[l].reshape(SLOTS, 8, 128, SLOT_COLS)
                .transpose(0, 2, 1, 3).reshape(SLOTS * 128, 8 * SLOT_COLS))
        m["t0n"] = np.ascontiguousarray(t0n.reshape(2 * SLOTS, 128).T)
        m["cwt"] = np.ascontiguousarray(
            cwt.reshape(2 * SLOTS, 128, O).transpose(1, 0, 2).reshape(128, 2 * SLOTS * O))
        in_maps.append(m)
    return in_maps


def kernel(z, T1, T2, T3, T0, C_w, C_b, mask):
    nc = _get_nc()
    in_maps = _prepare_in_maps(z, T1, T2, T3, T0, C_w, mask)
    res = run_bass_kernel_spmd(nc, in_maps, core_ids=list(range(N_CORES)))
    total = np.zeros((O, B), np.float32)
    for c in range(N_CORES):
        total += res.results[c]["out"]
    C_b = np.asarray(C_b, dtype=np.float32)
    return (total.T + C_b).astype(np.float32)
